# revision 1
# baseline (speedup 1.0000x reference)
"""Structured butterfly kernel, bf16 datapath (fp32 PSUM accumulation).

x is cast to bf16 AND pre-transposed on host, so the device does only
contiguous DMA loads.  Stages 0-6 (128x128 block-diag) run as
data-stationary bf16 matmuls into one 2-bank PSUM tile per subtile, so
the y drain is a single merged scatter copy; stages 7-9 as a PE
transpose pass + bf16 matmuls against 16x block-diag(8x8) weights.
Drain assignment (HW-measured best): y and o.q0 alternate ACT/DVE by
subtile parity, the z drain is pinned to DVE (bf16 runs 2x there, 1x
on ACT's InstActivation), and o.q1 is pinned to ACT as payback.  Loads
ride the gpsimd SWDGE queue, stores the SP ring.  The output leaves
the device in (g, hp, j16) column order; the host un-permutes with a
cheap reshape/transpose and upcasts bf16 -> f32.
"""

import numpy as np
import ml_dtypes

import concourse.bacc as bacc
import concourse.mybir as mybir
import concourse.tile as tile
from concourse.bass_utils import run_bass_kernel_spmd
from concourse.masks import make_identity

N_CORES = 8
BATCH = 32768
DIM = 1024
STAGES = 10
P = 128
ROWS_PER_CORE = BATCH // N_CORES          # 4096
R_SUPER = 1024                            # rows per load DMA
N_SUPER = ROWS_PER_CORE // R_SUPER        # 4
N_TILES = ROWS_PER_CORE // P              # 32
N_CHUNKS = DIM // P                       # 8
F32 = mybir.dt.float32
BF16 = mybir.dt.bfloat16

_NC = {}


def _stage_product(angles: np.ndarray, stages) -> np.ndarray:
    B = np.eye(DIM, dtype=np.float64)
    k = np.arange(DIM)
    for s in stages:
        stride = 1 << s
        b = k // (2 * stride)
        j = k % stride
        h = (k >> s) & 1
        th = angles[s].astype(np.float64)[b * stride + j]
        C = np.cos(th)
        S = np.where(h == 0, -np.sin(th), np.sin(th))
        B = C[:, None] * B + S[:, None] * B[k ^ stride]
    return B


def _build_weights(angles: np.ndarray):
    """Returns (WA [128, 1024], WB [128, 1024]) bf16.

    WA: per-chunk transposed stage-0..6 product (y keeps natural
    column order h*128 + c).
    WB: for zt partition order (h, j16) of group g (c = g*16 + j16),
    moving columns in (hp, j16) order.
    """
    B_lo = _stage_product(angles, range(7))
    B_hi = _stage_product(angles, range(7, 10))

    WA = np.zeros((P, N_CHUNKS * P), dtype=np.float64)
    for h in range(N_CHUNKS):
        blk = B_lo[h * P:(h + 1) * P, h * P:(h + 1) * P]
        WA[:, h * P:(h + 1) * P] = blk.T

    jj = np.arange(P)
    H = np.zeros((P, 8, 8), dtype=np.float64)
    for hp in range(8):
        for h in range(8):
            H[:, hp, h] = B_hi[hp * P + jj, h * P + jj]

    WB = np.zeros((P, N_CHUNKS * P), dtype=np.float64)
    for g in range(8):
        blk = np.zeros((P, P), dtype=np.float64)
        for j16 in range(16):
            j = 16 * g + j16
            for h in range(8):
                for hp in range(8):
                    blk[h * 16 + j16, hp * 16 + j16] = H[j, hp, h]
        WB[:, g * P:(g + 1) * P] = blk
    return (WA.astype(ml_dtypes.bfloat16), WB.astype(ml_dtypes.bfloat16))


def _build_nc(repeat: int = 1):
    nc = bacc.Bacc(
        "TRN2", target_bir_lowering=False, debug=False, num_devices=N_CORES
    )
    # x arrives pre-transposed: xT[d, r] = x[r, d]
    x_in = nc.dram_tensor(
        "x", [DIM, ROWS_PER_CORE], BF16, kind="ExternalInput"
    ).ap()
    wa_in = nc.dram_tensor("wa", [P, DIM], BF16, kind="ExternalInput").ap()
    wb_in = nc.dram_tensor("wb", [P, DIM], BF16, kind="ExternalInput").ap()
    out = nc.dram_tensor(
        "out", [ROWS_PER_CORE, DIM], BF16, kind="ExternalOutput"
    ).ap()

    # [i, h, r] view of xT: row d = h*128 + i
    xv = x_in.rearrange("(h i) r -> i h r", i=P)

    with tile.TileContext(nc) as tc:
        from contextlib import ExitStack

        with ExitStack() as ctx:
            const = ctx.enter_context(tc.tile_pool(name="const", bufs=1))
            ident = const.tile([P, P], BF16)
            make_identity(nc, ident)

            # 4+2+2 = 8 banks: mma tiles span 2 banks each so all 8
            # MM-A matmuls share one tile and y drains in a single op.
            mma = ctx.enter_context(
                tc.tile_pool(name="mma", bufs=2, space="PSUM")
            )
            tp2 = ctx.enter_context(
                tc.tile_pool(name="tp2", bufs=2, space="PSUM")
            )
            mmb = ctx.enter_context(
                tc.tile_pool(name="mmb", bufs=2, space="PSUM")
            )

            # Consume identity on PE early (single-wait discipline) and
            # trigger the ACT function-table load during startup.
            warm = tp2.tile([P, P], BF16, tag="pt2")
            nc.tensor.transpose(warm[:], ident[:], ident[:])
            warm_act = const.tile([P, 1], BF16)
            nc.scalar.copy(out=warm_act[:], in_=ident[:, 0:1])

            wa_sb = const.tile([P, DIM], BF16)
            nc.sync.dma_start(wa_sb[:], wa_in[:])
            wb_sb = const.tile([P, DIM], BF16)
            nc.sync.dma_start(wb_sb[:], wb_in[:])

            xt_pool = ctx.enter_context(tc.tile_pool(name="xt", bufs=3))
            y_pool = ctx.enter_context(tc.tile_pool(name="y", bufs=4))
            zt_pool = ctx.enter_context(tc.tile_pool(name="zt", bufs=4))
            o_pool = ctx.enter_context(tc.tile_pool(name="o", bufs=4))

            H4 = P * 4

            def cp(use_dve, out_ap, in_ap):
                if use_dve:
                    nc.vector.tensor_copy(out=out_ap, in_=in_ap)
                else:
                    nc.scalar.copy(out=out_ap, in_=in_ap)

            def pass_a(xt, rbase, dve):
                """MM-A for one subtile -> (g, h, j16)-ordered bf16 y.

                The h-regrouping scatter rides the y drain (matmul
                stationary APs must be 2D, so it cannot fold into T2).
                """
                y_t = y_pool.tile([P, DIM], BF16, tag="y")
                # Out dims iterated (g, h, j): h+j merge into 128-elem
                # contiguous write runs (vs 16-elem in (h, g, j) order).
                y_scatter = y_t[:].rearrange("p (g h j) -> p g h j", g=8, h=8)
                # 2-bank mma tile: all 8 matmuls land in one tile so the
                # y drain is a single merged op.
                bank_a = mma.tile([P, DIM], F32, tag="pa")
                for h in range(8):
                    nc.tensor.matmul(
                        bank_a[:, h * P : (h + 1) * P],
                        xt[:, h, rbase : rbase + P],
                        wa_sb[:, h * P : (h + 1) * P],
                        start=True,
                        stop=True,
                    )
                cp(
                    dve,
                    y_scatter[:],
                    bank_a[:].rearrange("p (h g j) -> p g h j", h=8, g=8),
                )
                return y_t

            def pass_b(y_t, row0, dve, k4=0):
                """T2 (gather-AP folds the (h,j16) regroup) + MM-B;
                drains contiguous; stores permuted bf16 output."""
                bank_t2 = tp2.tile([P, DIM], BF16, tag="pt2")
                zt_q = zt_pool.tile([P, DIM], BF16, tag="zt")
                for g in range(8):
                    nc.tensor.transpose(
                        bank_t2[:, g * P : (g + 1) * P],
                        y_t[:, g * P : (g + 1) * P],
                        ident[:],
                    )
                # z always drains on DVE: bf16->bf16 runs at 2x there,
                # but at 1x on ACT's InstActivation.
                cp(True, zt_q[:], bank_t2[:])

                o_t = o_pool.tile([P, DIM], BF16, tag="o")
                for q in range(2):
                    bank_b = mmb.tile([P, H4], F32, tag="pb")
                    for gg in range(4):
                        g = 4 * q + gg
                        nc.tensor.matmul(
                            bank_b[:, gg * P : (gg + 1) * P],
                            zt_q[:, g * P : (g + 1) * P],
                            wb_sb[:, g * P : (g + 1) * P],
                            start=True,
                            stop=True,
                        )
                    # DVE-subtiles hand their q1 o-half to ACT, paying
                    # back the z drains DVE absorbs from ACT-subtiles.
                    o_dve = dve if q == 0 else False
                    cp(o_dve, o_t[:, q * H4 : (q + 1) * H4], bank_b[:])
                nc.sync.dma_start(out[row0 : row0 + P, :], o_t[:])

            def full_pass():
                # 1-deep software pipeline: pass_b of subtile k-1 runs
                # while pass_a of subtile k fills; each subtile's drains
                # all ride one engine, alternating ACT/DVE by parity.
                k = 0
                pending = []
                for st in range(N_SUPER):
                    r0 = st * R_SUPER
                    # xt[i, h, r] = x[r0 + r, h*128 + i]: one big DMA on
                    # the gpsimd SWDGE queue.
                    xt = xt_pool.tile([P, N_CHUNKS, R_SUPER], BF16, tag="xt")
                    half = R_SUPER // 2
                    nc.gpsimd.dma_start(
                        xt[:, :, 0:half], xv[:, :, r0 : r0 + half]
                    )
                    nc.gpsimd.dma_start(
                        xt[:, :, half:R_SUPER],
                        xv[:, :, r0 + half : r0 + R_SUPER],
                    )
                    for rr in range(R_SUPER // P):
                        dve = k % 2 == 0
                        y_t = pass_a(xt, rr * P, dve)
                        pending.append((y_t, r0 + rr * P, dve, (k // 2) % 2))
                        if len(pending) > 1:
                            pass_b(*pending.pop(0))
                        k += 1
                for args in pending:
                    pass_b(*args)

            if repeat > 1:
                # Hardware loop: program size stays constant so large
                # repeat counts (for slope timing) compile fast.
                with tc.For_i(0, repeat):
                    full_pass()
            else:
                full_pass()

    nc.compile()
    return nc


def _get_nc(repeat: int = 1):
    if repeat not in _NC:
        _NC[repeat] = _build_nc(repeat)
    return _NC[repeat]


def prepare_in_maps(x, angles):
    WA, WB = _build_weights(angles)
    xb = x.astype(ml_dtypes.bfloat16)
    shards = xb.reshape(N_CORES, ROWS_PER_CORE, DIM)
    return [
        {
            "x": np.ascontiguousarray(shards[i].T),
            "wa": WA,
            "wb": WB,
        }
        for i in range(N_CORES)
    ]


def unpermute(out_dev: np.ndarray) -> np.ndarray:
    """Device col (g, hp, j16) -> true col (hp, g, j16)."""
    b = out_dev.shape[0]
    return np.ascontiguousarray(
        out_dev.reshape(b, 8, 8, 16).transpose(0, 2, 1, 3).reshape(b, DIM)
    )


def host_ref(x, angles):
    B = _stage_product(angles, range(STAGES))
    return x.astype(np.float64) @ B.T


def kernel(x: np.ndarray, angles: np.ndarray) -> np.ndarray:
    x = np.ascontiguousarray(np.asarray(x, dtype=np.float32))
    angles = np.asarray(angles, dtype=np.float32)
    assert x.shape == (BATCH, DIM), x.shape

    in_maps = prepare_in_maps(x, angles)

    nc = _get_nc()
    res = run_bass_kernel_spmd(nc, in_maps, list(range(N_CORES)))
    out = np.concatenate(
        [unpermute(res.results[i]["out"].astype(np.float32))
         for i in range(N_CORES)],
        axis=0,
    )
    return out



# revision 7
# speedup vs baseline: 19.3103x; 19.3103x over previous
"""Structured butterfly kernel, bf16 datapath (fp32 PSUM accumulation).

x is cast to bf16 AND pre-transposed on host, so the device does only
contiguous DMA loads.  Stages 0-6 (128x128 block-diag) run as
data-stationary bf16 matmuls into one 2-bank PSUM tile per subtile, so
the y drain is a single merged scatter copy; stages 7-9 as a PE
transpose pass + bf16 matmuls against 16x block-diag(8x8) weights.
Drain assignment (HW-measured best): y and o.q0 alternate ACT/DVE by
subtile parity, the z drain is pinned to DVE (bf16 runs 2x there, 1x
on ACT's InstActivation), and o.q1 is pinned to ACT as payback.  Loads
ride the gpsimd SWDGE queue, stores the SP ring.  The output leaves
the device in (g, hp, j16) column order; the host un-permutes with a
cheap reshape/transpose and upcasts bf16 -> f32.
"""

import numpy as np
import ml_dtypes

import concourse.bacc as bacc
import concourse.mybir as mybir
import concourse.tile as tile
from concourse.bass_utils import run_bass_kernel_spmd
from concourse.masks import make_identity

N_CORES = 8
BATCH = 32768
DIM = 1024
STAGES = 10
P = 128
ROWS_PER_CORE = BATCH // N_CORES          # 4096
R_SUPER = 1024                            # rows per load DMA
N_SUPER = ROWS_PER_CORE // R_SUPER        # 4
N_TILES = ROWS_PER_CORE // P              # 32
N_CHUNKS = DIM // P                       # 8
F32 = mybir.dt.float32
BF16 = mybir.dt.bfloat16

_NC = {}


def _stage_product(angles: np.ndarray, stages) -> np.ndarray:
    B = np.eye(DIM, dtype=np.float64)
    k = np.arange(DIM)
    for s in stages:
        stride = 1 << s
        b = k // (2 * stride)
        j = k % stride
        h = (k >> s) & 1
        th = angles[s].astype(np.float64)[b * stride + j]
        C = np.cos(th)
        S = np.where(h == 0, -np.sin(th), np.sin(th))
        B = C[:, None] * B + S[:, None] * B[k ^ stride]
    return B


def _build_weights(angles: np.ndarray):
    """Returns (WA [128, 1024], WB [128, 1024]) bf16.

    WA: per-chunk transposed stage-0..6 product (y keeps natural
    column order h*128 + c).
    WB: for zt partition order (h, j16) of group g (c = g*16 + j16),
    moving columns in (hp, j16) order.
    """
    B_lo = _stage_product(angles, range(7))
    B_hi = _stage_product(angles, range(7, 10))

    WA = np.zeros((P, N_CHUNKS * P), dtype=np.float64)
    for h in range(N_CHUNKS):
        blk = B_lo[h * P:(h + 1) * P, h * P:(h + 1) * P]
        WA[:, h * P:(h + 1) * P] = blk.T

    jj = np.arange(P)
    H = np.zeros((P, 8, 8), dtype=np.float64)
    for hp in range(8):
        for h in range(8):
            H[:, hp, h] = B_hi[hp * P + jj, h * P + jj]

    WB = np.zeros((P, N_CHUNKS * P), dtype=np.float64)
    for g in range(8):
        blk = np.zeros((P, P), dtype=np.float64)
        for j16 in range(16):
            j = 16 * g + j16
            for h in range(8):
                for hp in range(8):
                    blk[h * 16 + j16, hp * 16 + j16] = H[j, hp, h]
        WB[:, g * P:(g + 1) * P] = blk
    return (WA.astype(ml_dtypes.bfloat16), WB.astype(ml_dtypes.bfloat16))


def _build_nc(repeat: int = 1):
    nc = bacc.Bacc(
        "TRN2", target_bir_lowering=False, debug=False, num_devices=N_CORES
    )
    # x arrives pre-transposed: xT[d, r] = x[r, d]
    x_in = nc.dram_tensor(
        "x", [DIM, ROWS_PER_CORE], BF16, kind="ExternalInput"
    ).ap()
    wa_in = nc.dram_tensor("wa", [P, DIM], BF16, kind="ExternalInput").ap()
    wb_in = nc.dram_tensor("wb", [P, DIM], BF16, kind="ExternalInput").ap()
    out = nc.dram_tensor(
        "out", [ROWS_PER_CORE, DIM], BF16, kind="ExternalOutput"
    ).ap()

    # [i, h, r] view of xT: row d = h*128 + i
    xv = x_in.rearrange("(h i) r -> i h r", i=P)

    with tile.TileContext(nc) as tc:
        from contextlib import ExitStack

        with ExitStack() as ctx:
            const = ctx.enter_context(tc.tile_pool(name="const", bufs=1))
            ident = const.tile([P, P], BF16)
            make_identity(nc, ident)

            # 4+2+2 = 8 banks: mma tiles span 2 banks each so all 8
            # MM-A matmuls share one tile and y drains in a single op.
            mma = ctx.enter_context(
                tc.tile_pool(name="mma", bufs=2, space="PSUM")
            )
            tp2 = ctx.enter_context(
                tc.tile_pool(name="tp2", bufs=2, space="PSUM")
            )
            mmb = ctx.enter_context(
                tc.tile_pool(name="mmb", bufs=2, space="PSUM")
            )

            # Consume identity on PE early (single-wait discipline) and
            # trigger the ACT function-table load during startup.
            warm = tp2.tile([P, P], BF16, tag="pt2")
            nc.tensor.transpose(warm[:], ident[:], ident[:])
            warm_act = const.tile([P, 1], BF16)
            nc.scalar.copy(out=warm_act[:], in_=ident[:, 0:1])

            wa_sb = const.tile([P, DIM], BF16)
            nc.sync.dma_start(wa_sb[:], wa_in[:])
            wb_sb = const.tile([P, DIM], BF16)
            nc.sync.dma_start(wb_sb[:], wb_in[:])

            xt_pool = ctx.enter_context(tc.tile_pool(name="xt", bufs=3))
            y_pool = ctx.enter_context(tc.tile_pool(name="y", bufs=4))
            zt_pool = ctx.enter_context(tc.tile_pool(name="zt", bufs=4))
            o_pool = ctx.enter_context(tc.tile_pool(name="o", bufs=4))

            H4 = P * 4

            def cp(use_dve, out_ap, in_ap):
                if use_dve:
                    nc.vector.tensor_copy(out=out_ap, in_=in_ap)
                else:
                    nc.scalar.copy(out=out_ap, in_=in_ap)

            def pass_a(xt, rbase, dve):
                """dve: route the y drain to DVE (else ACT)."""
                """MM-A for one subtile -> (g, h, j16)-ordered bf16 y.

                The h-regrouping scatter rides the y drain (matmul
                stationary APs must be 2D, so it cannot fold into T2).
                """
                y_t = y_pool.tile([P, DIM], BF16, tag="y")
                # Out dims iterated (g, h, j): h+j merge into 128-elem
                # contiguous write runs (vs 16-elem in (h, g, j) order).
                y_scatter = y_t[:].rearrange("p (g h j) -> p g h j", g=8, h=8)
                # 2-bank mma tile: all 8 matmuls land in one tile so the
                # y drain is a single merged op.
                bank_a = mma.tile([P, DIM], F32, tag="pa")
                for h in range(8):
                    nc.tensor.matmul(
                        bank_a[:, h * P : (h + 1) * P],
                        xt[:, h, rbase : rbase + P],
                        wa_sb[:, h * P : (h + 1) * P],
                        start=True,
                        stop=True,
                    )
                cp(
                    dve,
                    y_scatter[:],
                    bank_a[:].rearrange("p (h g j) -> p g h j", h=8, g=8),
                )
                return y_t

            def pass_b(y_t, row0, q0_dve, k4=0):
                """T2 (gather-AP folds the (h,j16) regroup) + MM-B;
                drains contiguous; stores permuted bf16 output."""
                bank_t2 = tp2.tile([P, DIM], BF16, tag="pt2")
                zt_q = zt_pool.tile([P, DIM], BF16, tag="zt")
                for g in range(8):
                    nc.tensor.transpose(
                        bank_t2[:, g * P : (g + 1) * P],
                        y_t[:, g * P : (g + 1) * P],
                        ident[:],
                    )
                # z always drains on DVE: bf16->bf16 runs at 2x there,
                # but at 1x on ACT's InstActivation.
                cp(True, zt_q[:], bank_t2[:])

                o_t = o_pool.tile([P, DIM], BF16, tag="o")
                for q in range(2):
                    bank_b = mmb.tile([P, H4], F32, tag="pb")
                    for gg in range(4):
                        g = 4 * q + gg
                        nc.tensor.matmul(
                            bank_b[:, gg * P : (gg + 1) * P],
                            zt_q[:, g * P : (g + 1) * P],
                            wb_sb[:, g * P : (g + 1) * P],
                            start=True,
                            stop=True,
                        )
                    # q0 occasionally rides DVE (LP-balanced share);
                    # q1 is pinned to ACT.
                    o_dve = q0_dve if q == 0 else False
                    cp(o_dve, o_t[:, q * H4 : (q + 1) * H4], bank_b[:])
                nc.sync.dma_start(out[row0 : row0 + P, :], o_t[:])

            def full_pass():
                # 1-deep software pipeline: pass_b of subtile k-1 runs
                # while pass_a of subtile k fills.  Drain routing is the
                # LP-balanced split: z always DVE (2x bf16), y on DVE for
                # even subtiles, q0 on DVE 3-of-8, q1 always ACT.
                k = 0
                pending = []
                for st in range(N_SUPER):
                    r0 = st * R_SUPER
                    # xt[i, h, r] = x[r0 + r, h*128 + i]: one big DMA on
                    # the gpsimd SWDGE queue.  First supertile loads a
                    # small leading slice so the pipeline starts ~2.5us
                    # earlier.
                    xt = xt_pool.tile([P, N_CHUNKS, R_SUPER], BF16, tag="xt")
                    half = R_SUPER // 2
                    if st == 0:
                        nc.gpsimd.dma_start(
                            xt[:, :, 0:P], xv[:, :, r0 : r0 + P]
                        )
                        nc.gpsimd.dma_start(
                            xt[:, :, P:half], xv[:, :, r0 + P : r0 + half]
                        )
                    else:
                        nc.gpsimd.dma_start(
                            xt[:, :, 0:half], xv[:, :, r0 : r0 + half]
                        )
                    nc.gpsimd.dma_start(
                        xt[:, :, half:R_SUPER],
                        xv[:, :, r0 + half : r0 + R_SUPER],
                    )
                    for rr in range(R_SUPER // P):
                        i8 = k % 8
                        y_dve = i8 % 2 == 0
                        q0_dve = i8 in (1, 3, 5)
                        y_t = pass_a(xt, rr * P, y_dve)
                        pending.append((y_t, r0 + rr * P, q0_dve, (k // 2) % 2))
                        if len(pending) > 1:
                            pass_b(*pending.pop(0))
                        k += 1
                for args in pending:
                    pass_b(*args)

            if repeat > 1:
                # Hardware loop: program size stays constant so large
                # repeat counts (for slope timing) compile fast.
                with tc.For_i(0, repeat):
                    full_pass()
            else:
                full_pass()

    nc.compile()
    return nc


def _get_nc(repeat: int = 1):
    if repeat not in _NC:
        _NC[repeat] = _build_nc(repeat)
    return _NC[repeat]


def prepare_in_maps(x, angles):
    WA, WB = _build_weights(angles)
    xb = x.astype(ml_dtypes.bfloat16)
    shards = xb.reshape(N_CORES, ROWS_PER_CORE, DIM)
    return [
        {
            "x": np.ascontiguousarray(shards[i].T),
            "wa": WA,
            "wb": WB,
        }
        for i in range(N_CORES)
    ]


def unpermute(out_dev: np.ndarray) -> np.ndarray:
    """Device col (g, hp, j16) -> true col (hp, g, j16)."""
    b = out_dev.shape[0]
    return np.ascontiguousarray(
        out_dev.reshape(b, 8, 8, 16).transpose(0, 2, 1, 3).reshape(b, DIM)
    )


def host_ref(x, angles):
    B = _stage_product(angles, range(STAGES))
    return x.astype(np.float64) @ B.T


def kernel(x: np.ndarray, angles: np.ndarray) -> np.ndarray:
    x = np.ascontiguousarray(np.asarray(x, dtype=np.float32))
    angles = np.asarray(angles, dtype=np.float32)
    assert x.shape == (BATCH, DIM), x.shape

    in_maps = prepare_in_maps(x, angles)

    nc = _get_nc()
    res = run_bass_kernel_spmd(nc, in_maps, list(range(N_CORES)))
    out = np.concatenate(
        [unpermute(res.results[i]["out"].astype(np.float32))
         for i in range(N_CORES)],
        axis=0,
    )
    return out



# revision 47
# speedup vs baseline: 19.7123x; 1.0208x over previous
"""Structured butterfly kernel, bf16 datapath (fp32 PSUM accumulation).

x is cast to bf16 AND pre-transposed on host, so the device does only
contiguous DMA loads.  Stages 0-6 (128x128 block-diag) run as
data-stationary bf16 matmuls into one 2-bank PSUM tile per subtile, so
the y drain is a single merged scatter copy; stages 7-9 as a PE
transpose pass + bf16 matmuls against 16x block-diag(8x8) weights.
Drain assignment (HW-measured best): y and o.q0 alternate ACT/DVE by
subtile parity, the z drain is pinned to DVE (bf16 runs 2x there, 1x
on ACT's InstActivation), and o.q1 is pinned to ACT as payback.  Loads
ride the gpsimd SWDGE queue, stores the SP ring.  The output leaves
the device in (g, hp, j16) column order; the host un-permutes with a
cheap reshape/transpose and upcasts bf16 -> f32.
"""

import os

import numpy as np
import ml_dtypes

import concourse.bacc as bacc
import concourse.mybir as mybir
import concourse.tile as tile
from concourse.bass_utils import run_bass_kernel_spmd
from concourse.masks import make_identity

N_CORES = 8
BATCH = 32768
DIM = 1024
STAGES = 10
P = 128
ROWS_PER_CORE = BATCH // N_CORES          # 4096
R_SUPER = 1024                            # rows per load DMA
N_SUPER = ROWS_PER_CORE // R_SUPER        # 4
N_TILES = ROWS_PER_CORE // P              # 32
N_CHUNKS = DIM // P                       # 8
R_BLK = int(__import__("os").environ.get("K_RBLK", "1024"))
N_BLK = ROWS_PER_CORE // R_BLK            # load blocks per core
F32 = mybir.dt.float32
BF16 = mybir.dt.bfloat16

_NC = {}

# Drain routing (subtile-index-mod-8 sets), overridable for A/B benching.
_Y_DVE = tuple(
    int(c) for c in os.environ.get("K_Y_DVE", "0246")
)
_Q0_DVE = tuple(
    int(c) for c in os.environ.get("K_Q0_DVE", "135")
)
_LOADSPLIT = os.environ.get("K_LOADSPLIT", "1") == "1"
# Timing-only probes: drop one component to locate the HW critical path.
# full | nostore | noload | nodrain | nope
_PROBE = os.environ.get("K_PROBE", "full")
# Load DMA ring: gp = gpsimd SWDGE, act = qActDynamicHW, sp = qSPDynamicHW
# (must be gp when _XDT casts: only SWDGE DMAs can convert dtypes)
_LOADQ = os.environ.get("K_LOADQ", "gp")
# DRAM dtype of x: f8 = float8_e3m4 cast to bf16 in the load DMA;
# f8sb = float8_e3m4 end-to-end (fp8 in SBUF, MMA reads fp8 moving
# operand directly — halves both DRAM AND SBUF-write traffic);
# bf16 = as-is.  f8 variants add ~1.3% input quantization error.
_XDT = os.environ.get("K_XDT", "f8sb")
# Output DRAM dtype: bf16 | f8cast (timing probe: whole output stored
# as e3m4 via gpsimd cast-DMA — exceeds the error budget, probe only)
_ODT = os.environ.get("K_ODT", "bf16")


def _stage_product(angles: np.ndarray, stages) -> np.ndarray:
    B = np.eye(DIM, dtype=np.float64)
    k = np.arange(DIM)
    for s in stages:
        stride = 1 << s
        b = k // (2 * stride)
        j = k % stride
        h = (k >> s) & 1
        th = angles[s].astype(np.float64)[b * stride + j]
        C = np.cos(th)
        S = np.where(h == 0, -np.sin(th), np.sin(th))
        B = C[:, None] * B + S[:, None] * B[k ^ stride]
    return B


def _build_weights(angles: np.ndarray):
    """Returns (WA [128, 1024], WB [128, 1024]) bf16.

    WA: per-chunk transposed stage-0..6 product (y keeps natural
    column order h*128 + c).
    WB: for zt partition order (h, j16) of group g (c = g*16 + j16),
    moving columns in (hp, j16) order.
    """
    B_lo = _stage_product(angles, range(7))
    B_hi = _stage_product(angles, range(7, 10))

    WA = np.zeros((P, N_CHUNKS * P), dtype=np.float64)
    for h in range(N_CHUNKS):
        blk = B_lo[h * P:(h + 1) * P, h * P:(h + 1) * P]
        WA[:, h * P:(h + 1) * P] = blk.T

    jj = np.arange(P)
    H = np.zeros((P, 8, 8), dtype=np.float64)
    for hp in range(8):
        for h in range(8):
            H[:, hp, h] = B_hi[hp * P + jj, h * P + jj]

    WB = np.zeros((P, N_CHUNKS * P), dtype=np.float64)
    for g in range(8):
        blk = np.zeros((P, P), dtype=np.float64)
        for j16 in range(16):
            j = 16 * g + j16
            for h in range(8):
                for hp in range(8):
                    blk[h * 16 + j16, hp * 16 + j16] = H[j, hp, h]
        WB[:, g * P:(g + 1) * P] = blk
    return (WA.astype(ml_dtypes.bfloat16), WB.astype(ml_dtypes.bfloat16))


def _build_nc(repeat: int = 1):
    nc = bacc.Bacc(
        "TRN2", target_bir_lowering=False, debug=False, num_devices=N_CORES
    )
    # x arrives pre-transposed AND load-tiled on host:
    # x_in[b, i, h, r] = x[b*R_BLK + r, h*128 + i], so each load DMA
    # reads one fully contiguous 512KB DRAM block (4KB runs per
    # partition, ascending addresses — DRAM-sequential).
    F8 = mybir.dt.float8e3
    x_in = nc.dram_tensor(
        "x",
        [N_BLK, P, N_CHUNKS, R_BLK],
        F8 if _XDT in ("f8", "f8sb") else BF16,
        kind="ExternalInput",
    ).ap()
    XT_SB = F8 if _XDT == "f8sb" else BF16
    wa_in = nc.dram_tensor("wa", [P, DIM], BF16, kind="ExternalInput").ap()
    wb_in = nc.dram_tensor("wb", [P, DIM], BF16, kind="ExternalInput").ap()
    out = nc.dram_tensor(
        "out",
        [ROWS_PER_CORE, DIM],
        F8 if _ODT == "f8cast" else BF16,
        kind="ExternalOutput",
    ).ap()



    with tile.TileContext(nc) as tc:
        from contextlib import ExitStack

        with ExitStack() as ctx:
            const = ctx.enter_context(tc.tile_pool(name="const", bufs=1))
            ident = const.tile([P, P], BF16)
            make_identity(nc, ident)

            # 4+2+2 = 8 banks: mma tiles span 2 banks each so all 8
            # MM-A matmuls share one tile and y drains in a single op.
            mma = ctx.enter_context(
                tc.tile_pool(name="mma", bufs=2, space="PSUM")
            )
            tp2 = ctx.enter_context(
                tc.tile_pool(name="tp2", bufs=2, space="PSUM")
            )
            mmb = ctx.enter_context(
                tc.tile_pool(name="mmb", bufs=2, space="PSUM")
            )

            # Consume identity on PE early (single-wait discipline) and
            # trigger the ACT function-table load during startup.
            warm = tp2.tile([P, P], BF16, tag="pt2")
            nc.tensor.transpose(warm[:], ident[:], ident[:])
            warm_act = const.tile([P, 1], BF16)
            nc.scalar.copy(out=warm_act[:], in_=ident[:, 0:1])

            wa_sb = const.tile([P, DIM], BF16)
            nc.sync.dma_start(wa_sb[:], wa_in[:])
            wb_sb = const.tile([P, DIM], BF16)
            nc.sync.dma_start(wb_sb[:], wb_in[:])

            xt_pool = ctx.enter_context(tc.tile_pool(name="xt", bufs=3))
            y_pool = ctx.enter_context(tc.tile_pool(name="y", bufs=4))
            zt_pool = ctx.enter_context(tc.tile_pool(name="zt", bufs=4))
            o_pool = ctx.enter_context(tc.tile_pool(name="o", bufs=4))

            H4 = P * 4

            def cp(use_dve, out_ap, in_ap):
                if _PROBE == "nodrain":
                    # keep the tile allocated with a 1-col copy
                    nc.gpsimd.memset(out_ap[:, 0:1], 0.0)
                    return
                if use_dve:
                    nc.vector.tensor_copy(out=out_ap, in_=in_ap)
                else:
                    nc.scalar.copy(out=out_ap, in_=in_ap)

            def pass_a(xt, rbase, dve):
                """dve: route the y drain to DVE (else ACT)."""
                """MM-A for one subtile -> (g, h, j16)-ordered bf16 y.

                The h-regrouping scatter rides the y drain (matmul
                stationary APs must be 2D, so it cannot fold into T2).
                """
                y_t = y_pool.tile([P, DIM], BF16, tag="y")
                # Out dims iterated (g, h, j): h+j merge into 128-elem
                # contiguous write runs (vs 16-elem in (h, g, j) order).
                y_scatter = y_t[:].rearrange("p (g h j) -> p g h j", g=8, h=8)
                # 2-bank mma tile: all 8 matmuls land in one tile so the
                # y drain is a single merged op.
                bank_a = mma.tile([P, DIM], F32, tag="pa")
                nh = 1 if _PROBE == "nope" else 8
                for h in range(nh):
                    nc.tensor.matmul(
                        bank_a[:, h * P : (h + 1) * P],
                        xt[:, h, rbase : rbase + P],
                        wa_sb[:, h * P : (h + 1) * P],
                        start=True,
                        stop=True,
                    )
                cp(
                    dve,
                    y_scatter[:],
                    bank_a[:].rearrange("p (h g j) -> p g h j", h=8, g=8),
                )
                return y_t

            def pass_b1(y_t):
                """T2 (gather-AP folds the (h,j16) regroup) + z drain."""
                bank_t2 = tp2.tile([P, DIM], BF16, tag="pt2")
                zt_q = zt_pool.tile([P, DIM], BF16, tag="zt")
                ng = 1 if _PROBE == "nope" else 8
                for g in range(ng):
                    nc.tensor.transpose(
                        bank_t2[:, g * P : (g + 1) * P],
                        y_t[:, g * P : (g + 1) * P],
                        ident[:],
                    )
                # z always drains on DVE: bf16->bf16 runs at 2x there,
                # but at 1x on ACT's InstActivation.
                cp(True, zt_q[:], bank_t2[:])
                return zt_q

            def pass_b2(zt_q, row0, q0_dve, k4=0):
                """MM-B; drains contiguous; stores permuted bf16 output.
                Runs 2 subtiles behind T2 so PE never waits on the z
                drain's PSUM->SBUF copy + semaphore round trip."""
                o_t = o_pool.tile([P, DIM], BF16, tag="o")
                for q in range(2):
                    bank_b = mmb.tile([P, H4], F32, tag="pb")
                    ngg = 1 if _PROBE == "nope" else 4
                    for gg in range(ngg):
                        g = 4 * q + gg
                        nc.tensor.matmul(
                            bank_b[:, gg * P : (gg + 1) * P],
                            zt_q[:, g * P : (g + 1) * P],
                            wb_sb[:, g * P : (g + 1) * P],
                            start=True,
                            stop=True,
                        )
                    # q0 occasionally rides DVE (LP-balanced share);
                    # q1 is pinned to ACT.
                    o_dve = q0_dve if q == 0 else False
                    cp(o_dve, o_t[:, q * H4 : (q + 1) * H4], bank_b[:])
                if _PROBE != "nostore":
                    store_eng = (
                        nc.gpsimd if _ODT == "f8cast" else nc.sync
                    )
                    store_eng.dma_start(out[row0 : row0 + P, :], o_t[:])

            load_eng = {
                "gp": nc.gpsimd,
                "act": nc.scalar,
                "sp": nc.sync,
            }[_LOADQ]

            def load_supertile(st):
                """Load one supertile's xt via contiguous DRAM blocks."""
                xt = xt_pool.tile([P, N_CHUNKS, R_SUPER], XT_SB, tag="xt")
                b0 = st * (R_SUPER // R_BLK)
                for j in range(R_SUPER // R_BLK):
                    load_eng.dma_start(
                        xt[:, :, j * R_BLK : (j + 1) * R_BLK],
                        x_in[b0 + j],
                    )
                return xt

            def dma_probe_pass():
                # loads and/or stores only — measures raw DMA rates.
                for st in range(N_SUPER):
                    r0 = st * R_SUPER
                    if _PROBE in ("dmaonly", "loadonly"):
                        load_supertile(st)
                    if _PROBE in ("dmaonly", "storeonly"):
                        for rr in range(R_SUPER // P):
                            row0 = r0 + rr * P
                            o_t = o_pool.tile([P, DIM], BF16, tag="o")
                            nc.gpsimd.memset(o_t[:, 0:1], 0.0)
                            store_eng = (
                                nc.gpsimd if _ODT == "f8cast" else nc.sync
                            )
                            store_eng.dma_start(
                                out[row0 : row0 + P, :], o_t[:]
                            )

            def full_pass():
                if _PROBE in ("dmaonly", "loadonly", "storeonly"):
                    return dma_probe_pass()
                # 1-deep software pipeline, pass_b(k-1) issued BEFORE
                # pass_a(k): engines are in-order, so the ready work
                # (T2/MM-B drains of subtile k-1) must enter each engine
                # stream ahead of ops gated on MM-A(k).  Drain routing
                # is the LP-balanced split: z always DVE (2x bf16), y on
                # DVE for even subtiles, q0 on DVE 3-of-8, q1 always ACT.
                k = 0
                pend = []
                for st in range(N_SUPER):
                    r0 = st * R_SUPER
                    if _PROBE == "noload":
                        xt = xt_pool.tile(
                            [P, N_CHUNKS, R_SUPER], XT_SB, tag="xt"
                        )
                        nc.gpsimd.memset(xt[:, 0, 0:1], 0.0)
                    else:
                        xt = load_supertile(st)
                    for rr in range(R_SUPER // P):
                        i8 = k % 8
                        y_dve = i8 in _Y_DVE
                        q0_dve = i8 in _Q0_DVE
                        if pend:
                            y_a, row_a, qd_a, k4_a = pend.pop(0)
                            zt_a = pass_b1(y_a)
                            y_t = pass_a(xt, rr * P, y_dve)
                            pass_b2(zt_a, row_a, qd_a, k4_a)
                        else:
                            y_t = pass_a(xt, rr * P, y_dve)
                        pend.append(
                            (y_t, r0 + rr * P, q0_dve, (k // 2) % 2)
                        )
                        k += 1
                for y_a, row_a, qd_a, k4_a in pend:
                    zt_a = pass_b1(y_a)
                    pass_b2(zt_a, row_a, qd_a, k4_a)

            if repeat > 1:
                # Hardware loop: program size stays constant so large
                # repeat counts (for slope timing) compile fast.
                with tc.For_i(0, repeat):
                    full_pass()
            else:
                full_pass()

    nc.compile()
    return nc


def _get_nc(repeat: int = 1):
    if repeat not in _NC:
        _NC[repeat] = _build_nc(repeat)
    return _NC[repeat]


def prepare_in_maps(x, angles):
    WA, WB = _build_weights(angles)
    xdt = (
        ml_dtypes.float8_e3m4
        if _XDT in ("f8", "f8sb")
        else ml_dtypes.bfloat16
    )
    xb = x.astype(xdt)
    # [core, b, i, h, r] = x[core*4096 + b*256 + r, h*128 + i]: each
    # (b) is one contiguous load block on the device.
    tiled = np.ascontiguousarray(
        xb.reshape(N_CORES, N_BLK, R_BLK, N_CHUNKS, P).transpose(
            0, 1, 4, 3, 2
        )
    )
    return [
        {"x": tiled[i], "wa": WA, "wb": WB}
        for i in range(N_CORES)
    ]


def unpermute(out_dev: np.ndarray) -> np.ndarray:
    """Device col (g, hp, j16) -> true col (hp, g, j16)."""
    b = out_dev.shape[0]
    return np.ascontiguousarray(
        out_dev.reshape(b, 8, 8, 16).transpose(0, 2, 1, 3).reshape(b, DIM)
    )


def host_ref(x, angles):
    B = _stage_product(angles, range(STAGES))
    return x.astype(np.float64) @ B.T


def kernel(x: np.ndarray, angles: np.ndarray) -> np.ndarray:
    x = np.ascontiguousarray(np.asarray(x, dtype=np.float32))
    angles = np.asarray(angles, dtype=np.float32)
    assert x.shape == (BATCH, DIM), x.shape

    in_maps = prepare_in_maps(x, angles)

    nc = _get_nc()
    res = run_bass_kernel_spmd(nc, in_maps, list(range(N_CORES)))
    out = np.concatenate(
        [unpermute(res.results[i]["out"].astype(np.float32))
         for i in range(N_CORES)],
        axis=0,
    )
    return out



# revision 62
# speedup vs baseline: 20.6930x; 1.0497x over previous
"""Structured butterfly kernel, bf16 datapath (fp32 PSUM accumulation).

x is cast to bf16 AND pre-transposed on host, so the device does only
contiguous DMA loads.  Stages 0-6 (128x128 block-diag) run as
data-stationary bf16 matmuls into one 2-bank PSUM tile per subtile, so
the y drain is a single merged scatter copy; stages 7-9 as a PE
transpose pass + bf16 matmuls against 16x block-diag(8x8) weights.
Drain assignment (HW-measured best): y and o.q0 alternate ACT/DVE by
subtile parity, the z drain is pinned to DVE (bf16 runs 2x there, 1x
on ACT's InstActivation), and o.q1 is pinned to ACT as payback.  Loads
ride the gpsimd SWDGE queue, stores the SP ring.  The output leaves
the device in (g, hp, j16) column order; the host un-permutes with a
cheap reshape/transpose and upcasts bf16 -> f32.
"""

import os

import numpy as np
import ml_dtypes

import concourse.bacc as bacc
import concourse.mybir as mybir
import concourse.tile as tile
from concourse.bass_utils import run_bass_kernel_spmd
from concourse.masks import make_identity

N_CORES = 8
BATCH = 32768
DIM = 1024
STAGES = 10
P = 128
ROWS_PER_CORE = BATCH // N_CORES          # 4096
R_SUPER = 1024                            # rows per load DMA
N_SUPER = ROWS_PER_CORE // R_SUPER        # 4
N_TILES = ROWS_PER_CORE // P              # 32
N_CHUNKS = DIM // P                       # 8
R_BLK = int(__import__("os").environ.get("K_RBLK", "1024"))
N_BLK = ROWS_PER_CORE // R_BLK            # load blocks per core
F32 = mybir.dt.float32
BF16 = mybir.dt.bfloat16

_NC = {}

# Drain routing (subtile-index-mod-8 sets), overridable for A/B benching.
_Y_DVE = tuple(
    int(c) for c in os.environ.get("K_Y_DVE", "0246")
)
_Q0_DVE = tuple(
    int(c) for c in os.environ.get("K_Q0_DVE", "135")
)
_LOADSPLIT = os.environ.get("K_LOADSPLIT", "1") == "1"
# Timing-only probes: drop one component to locate the HW critical path.
# full | nostore | noload | nodrain | nope
_PROBE = os.environ.get("K_PROBE", "full")
# Load DMA ring: gp = gpsimd SWDGE, act = qActDynamicHW, sp = qSPDynamicHW
# (must be gp when _XDT casts: only SWDGE DMAs can convert dtypes)
_LOADQ = os.environ.get("K_LOADQ", "gp")
# DRAM dtype of x: f8 = float8_e3m4 cast to bf16 in the load DMA;
# f8sb = float8_e3m4 end-to-end (fp8 in SBUF, MMA reads fp8 moving
# operand directly — halves both DRAM AND SBUF-write traffic);
# bf16 = as-is.  f8 variants add ~1.3% input quantization error.
_XDT = os.environ.get("K_XDT", "f8sb")
# Output DRAM dtype: rsplit = even subtiles bf16 + odd subtiles e3m4
# (row split: full-width DMAs, total err ~1.7%); split = column split
# (slow: halves store descriptor sizes); bf16 = all bf16; f8cast
# (probe only: all e3m4, exceeds the error budget)
# NOTE: e3m4 engine-writes are broken on TRN2 HW (DVE wedges the core,
# ACT is ~4x slow) — only bf16 output is viable.
_ODT = os.environ.get("K_ODT", "bf16")


def _stage_product(angles: np.ndarray, stages) -> np.ndarray:
    B = np.eye(DIM, dtype=np.float64)
    k = np.arange(DIM)
    for s in stages:
        stride = 1 << s
        b = k // (2 * stride)
        j = k % stride
        h = (k >> s) & 1
        th = angles[s].astype(np.float64)[b * stride + j]
        C = np.cos(th)
        S = np.where(h == 0, -np.sin(th), np.sin(th))
        B = C[:, None] * B + S[:, None] * B[k ^ stride]
    return B


def _build_weights(angles: np.ndarray):
    """Returns (WA [128, 1024], WB [128, 1024]) bf16.

    WA: per-chunk transposed stage-0..6 product (y keeps natural
    column order h*128 + c).
    WB: for zt partition order (h, j16) of group g (c = g*16 + j16),
    moving columns in (hp, j16) order.
    """
    B_lo = _stage_product(angles, range(7))
    B_hi = _stage_product(angles, range(7, 10))

    WA = np.zeros((P, N_CHUNKS * P), dtype=np.float64)
    for h in range(N_CHUNKS):
        blk = B_lo[h * P:(h + 1) * P, h * P:(h + 1) * P]
        WA[:, h * P:(h + 1) * P] = blk.T

    jj = np.arange(P)
    H = np.zeros((P, 8, 8), dtype=np.float64)
    for hp in range(8):
        for h in range(8):
            H[:, hp, h] = B_hi[hp * P + jj, h * P + jj]

    WB = np.zeros((P, N_CHUNKS * P), dtype=np.float64)
    for g in range(8):
        blk = np.zeros((P, P), dtype=np.float64)
        for j16 in range(16):
            j = 16 * g + j16
            for h in range(8):
                for hp in range(8):
                    blk[h * 16 + j16, hp * 16 + j16] = H[j, hp, h]
        WB[:, g * P:(g + 1) * P] = blk
    return (WA.astype(ml_dtypes.bfloat16), WB.astype(ml_dtypes.bfloat16))


def _build_nc(repeat: int = 1):
    nc = bacc.Bacc(
        "TRN2", target_bir_lowering=False, debug=False, num_devices=N_CORES
    )
    # x arrives pre-transposed AND load-tiled on host:
    # x_in[b, i, h, r] = x[b*R_BLK + r, h*128 + i], so each load DMA
    # reads one fully contiguous 512KB DRAM block (4KB runs per
    # partition, ascending addresses — DRAM-sequential).
    F8 = mybir.dt.float8e3
    x_in = nc.dram_tensor(
        "x",
        [N_BLK, P, N_CHUNKS, R_BLK],
        F8 if _XDT in ("f8", "f8sb") else BF16,
        kind="ExternalInput",
    ).ap()
    XT_SB = F8 if _XDT == "f8sb" else BF16
    wa_in = nc.dram_tensor("wa", [P, DIM], BF16, kind="ExternalInput").ap()
    wb_in = nc.dram_tensor("wb", [P, DIM], BF16, kind="ExternalInput").ap()
    if _ODT == "rsplit":
        out = nc.dram_tensor(
            "out", [ROWS_PER_CORE // 2, DIM], BF16, kind="ExternalOutput"
        ).ap()
        out8 = nc.dram_tensor(
            "out8", [ROWS_PER_CORE // 2, DIM], F8, kind="ExternalOutput"
        ).ap()
    elif _ODT == "split":
        out = nc.dram_tensor(
            "out", [ROWS_PER_CORE, DIM // 2], BF16, kind="ExternalOutput"
        ).ap()
        out8 = nc.dram_tensor(
            "out8", [ROWS_PER_CORE, DIM // 2], F8, kind="ExternalOutput"
        ).ap()
    else:
        out = nc.dram_tensor(
            "out",
            [ROWS_PER_CORE, DIM],
            F8 if _ODT == "f8cast" else BF16,
            kind="ExternalOutput",
        ).ap()
        out8 = None



    with tile.TileContext(nc) as tc:
        from contextlib import ExitStack

        with ExitStack() as ctx:
            const = ctx.enter_context(tc.tile_pool(name="const", bufs=1))
            ident = const.tile([P, P], BF16)
            make_identity(nc, ident)

            # 4+2+2 = 8 banks: mma tiles span 2 banks each so all 8
            # MM-A matmuls share one tile and y drains in a single op.
            mma = ctx.enter_context(
                tc.tile_pool(name="mma", bufs=2, space="PSUM")
            )
            tp2 = ctx.enter_context(
                tc.tile_pool(name="tp2", bufs=2, space="PSUM")
            )
            mmb = ctx.enter_context(
                tc.tile_pool(name="mmb", bufs=2, space="PSUM")
            )

            # Consume identity on PE early (single-wait discipline) and
            # trigger the ACT function-table load during startup.
            warm = tp2.tile([P, P], BF16, tag="pt2")
            nc.tensor.transpose(warm[:], ident[:], ident[:])
            warm_act = const.tile([P, 1], BF16)
            nc.scalar.copy(out=warm_act[:], in_=ident[:, 0:1])

            wa_sb = const.tile([P, DIM], BF16)
            nc.sync.dma_start(wa_sb[:], wa_in[:])
            wb_sb = const.tile([P, DIM], BF16)
            nc.sync.dma_start(wb_sb[:], wb_in[:])

            xt_pool = ctx.enter_context(tc.tile_pool(name="xt", bufs=3))
            y_pool = ctx.enter_context(tc.tile_pool(name="y", bufs=4))
            zt_pool = ctx.enter_context(tc.tile_pool(name="zt", bufs=4))
            o_pool = ctx.enter_context(tc.tile_pool(name="o", bufs=4))
            o8_pool = ctx.enter_context(tc.tile_pool(name="o8", bufs=4))

            H4 = P * 4

            def cp(use_dve, out_ap, in_ap):
                if _PROBE == "nodrain":
                    # keep the tile allocated with a 1-col copy
                    nc.gpsimd.memset(out_ap[:, 0:1], 0.0)
                    return
                if use_dve:
                    nc.vector.tensor_copy(out=out_ap, in_=in_ap)
                else:
                    nc.scalar.copy(out=out_ap, in_=in_ap)

            def pass_a(xt, rbase, dve):
                """dve: route the y drain to DVE (else ACT)."""
                """MM-A for one subtile -> (g, h, j16)-ordered bf16 y.

                The h-regrouping scatter rides the y drain (matmul
                stationary APs must be 2D, so it cannot fold into T2).
                """
                y_t = y_pool.tile([P, DIM], BF16, tag="y")
                # Out dims iterated (g, h, j): h+j merge into 128-elem
                # contiguous write runs (vs 16-elem in (h, g, j) order).
                y_scatter = y_t[:].rearrange("p (g h j) -> p g h j", g=8, h=8)
                # 2-bank mma tile: all 8 matmuls land in one tile so the
                # y drain is a single merged op.
                bank_a = mma.tile([P, DIM], F32, tag="pa")
                nh = 1 if _PROBE == "nope" else 8
                for h in range(nh):
                    nc.tensor.matmul(
                        bank_a[:, h * P : (h + 1) * P],
                        xt[:, h, rbase : rbase + P],
                        wa_sb[:, h * P : (h + 1) * P],
                        start=True,
                        stop=True,
                    )
                cp(
                    dve,
                    y_scatter[:],
                    bank_a[:].rearrange("p (h g j) -> p g h j", h=8, g=8),
                )
                return y_t

            def pass_b1(y_t):
                """T2 (gather-AP folds the (h,j16) regroup) + z drain."""
                bank_t2 = tp2.tile([P, DIM], BF16, tag="pt2")
                zt_q = zt_pool.tile([P, DIM], BF16, tag="zt")
                ng = 1 if _PROBE == "nope" else 8
                for g in range(ng):
                    nc.tensor.transpose(
                        bank_t2[:, g * P : (g + 1) * P],
                        y_t[:, g * P : (g + 1) * P],
                        ident[:],
                    )
                # z always drains on DVE: bf16->bf16 runs at 2x there,
                # but at 1x on ACT's InstActivation.
                cp(True, zt_q[:], bank_t2[:])
                return zt_q

            def pass_b2(zt_q, row0, q0_dve, f8_row=False, k4=0):
                """MM-B; drains contiguous; stores permuted output.
                Under rsplit, f8_row subtiles drain+store as e3m4."""
                split = _ODT == "split"
                rsplit = _ODT == "rsplit"
                if rsplit and f8_row:
                    o_t = None
                    o_8 = o8_pool.tile([P, DIM], F8, tag="o8")
                elif split:
                    o_t = o_pool.tile([P, H4], BF16, tag="o")
                    o_8 = o8_pool.tile([P, H4], F8, tag="o8")
                else:
                    o_t = o_pool.tile([P, DIM], BF16, tag="o")
                    o_8 = None
                for q in range(2):
                    bank_b = mmb.tile([P, H4], F32, tag="pb")
                    ngg = 1 if _PROBE == "nope" else 4
                    for gg in range(ngg):
                        g = 4 * q + gg
                        nc.tensor.matmul(
                            bank_b[:, gg * P : (gg + 1) * P],
                            zt_q[:, g * P : (g + 1) * P],
                            wb_sb[:, g * P : (g + 1) * P],
                            start=True,
                            stop=True,
                        )
                    # q0 occasionally rides DVE (LP-balanced share);
                    # q1 is pinned to ACT.  f8 rows: engine set by env
                    # (probing which engine converts to e3m4 faster).
                    if rsplit and f8_row:
                        o_dve = os.environ.get("K_F8ENG", "dve") == "dve"
                    else:
                        o_dve = q0_dve if q == 0 else False
                    if split:
                        dst = o_t[:] if q == 0 else o_8[:]
                    elif rsplit and f8_row:
                        dst = o_8[:, q * H4 : (q + 1) * H4]
                    else:
                        dst = o_t[:, q * H4 : (q + 1) * H4]
                    cp(o_dve, dst, bank_b[:])
                if _PROBE != "nostore":
                    if rsplit:
                        half_row = (row0 // (2 * P)) * P
                        if f8_row:
                            nc.sync.dma_start(
                                out8[half_row : half_row + P, :], o_8[:]
                            )
                        else:
                            nc.sync.dma_start(
                                out[half_row : half_row + P, :], o_t[:]
                            )
                    elif split:
                        nc.sync.dma_start(
                            out[row0 : row0 + P, :], o_t[:]
                        )
                        nc.sync.dma_start(
                            out8[row0 : row0 + P, :], o_8[:]
                        )
                    else:
                        store_eng = (
                            nc.gpsimd if _ODT == "f8cast" else nc.sync
                        )
                        store_eng.dma_start(
                            out[row0 : row0 + P, :], o_t[:]
                        )

            load_eng = {
                "gp": nc.gpsimd,
                "act": nc.scalar,
                "sp": nc.sync,
            }[_LOADQ]

            def load_supertile(st):
                """Load one supertile's xt via contiguous DRAM blocks."""
                xt = xt_pool.tile([P, N_CHUNKS, R_SUPER], XT_SB, tag="xt")
                b0 = st * (R_SUPER // R_BLK)
                for j in range(R_SUPER // R_BLK):
                    load_eng.dma_start(
                        xt[:, :, j * R_BLK : (j + 1) * R_BLK],
                        x_in[b0 + j],
                    )
                return xt

            def dma_probe_pass():
                # loads and/or stores only — measures raw DMA rates.
                for st in range(N_SUPER):
                    r0 = st * R_SUPER
                    if _PROBE in ("dmaonly", "loadonly"):
                        load_supertile(st)
                    if _PROBE in ("dmaonly", "storeonly"):
                        for rr in range(R_SUPER // P):
                            row0 = r0 + rr * P
                            if _ODT == "rsplit":
                                half_row = (row0 // (2 * P)) * P
                                if (row0 // P) % 2 == 1:
                                    o_8 = o8_pool.tile(
                                        [P, DIM], F8, tag="o8"
                                    )
                                    nc.gpsimd.memset(o_8[:, 0:1], 0.0)
                                    nc.sync.dma_start(
                                        out8[half_row : half_row + P, :],
                                        o_8[:],
                                    )
                                else:
                                    o_t = o_pool.tile(
                                        [P, DIM], BF16, tag="o"
                                    )
                                    nc.gpsimd.memset(o_t[:, 0:1], 0.0)
                                    nc.sync.dma_start(
                                        out[half_row : half_row + P, :],
                                        o_t[:],
                                    )
                            elif _ODT == "split":
                                o_t = o_pool.tile([P, H4], BF16, tag="o")
                                o_8 = o8_pool.tile([P, H4], F8, tag="o8")
                                nc.gpsimd.memset(o_t[:, 0:1], 0.0)
                                nc.gpsimd.memset(o_8[:, 0:1], 0.0)
                                nc.sync.dma_start(
                                    out[row0 : row0 + P, :], o_t[:]
                                )
                                nc.sync.dma_start(
                                    out8[row0 : row0 + P, :], o_8[:]
                                )
                            else:
                                o_t = o_pool.tile([P, DIM], BF16, tag="o")
                                nc.gpsimd.memset(o_t[:, 0:1], 0.0)
                                store_eng = (
                                    nc.gpsimd
                                    if _ODT == "f8cast"
                                    else nc.sync
                                )
                                store_eng.dma_start(
                                    out[row0 : row0 + P, :], o_t[:]
                                )

            def full_pass():
                if _PROBE in ("dmaonly", "loadonly", "storeonly"):
                    return dma_probe_pass()
                # 1-deep software pipeline, pass_b(k-1) issued BEFORE
                # pass_a(k): engines are in-order, so the ready work
                # (T2/MM-B drains of subtile k-1) must enter each engine
                # stream ahead of ops gated on MM-A(k).  Drain routing
                # is the LP-balanced split: z always DVE (2x bf16), y on
                # DVE for even subtiles, q0 on DVE 3-of-8, q1 always ACT.
                k = 0
                pend = []
                for st in range(N_SUPER):
                    r0 = st * R_SUPER
                    if _PROBE == "noload":
                        xt = xt_pool.tile(
                            [P, N_CHUNKS, R_SUPER], XT_SB, tag="xt"
                        )
                        nc.gpsimd.memset(xt[:, 0, 0:1], 0.0)
                    else:
                        xt = load_supertile(st)
                    for rr in range(R_SUPER // P):
                        i8 = k % 8
                        y_dve = i8 in _Y_DVE
                        q0_dve = i8 in _Q0_DVE
                        if pend:
                            y_a, row_a, qd_a, f8_a = pend.pop(0)
                            zt_a = pass_b1(y_a)
                            y_t = pass_a(xt, rr * P, y_dve)
                            pass_b2(zt_a, row_a, qd_a, f8_a)
                        else:
                            y_t = pass_a(xt, rr * P, y_dve)
                        pend.append(
                            (y_t, r0 + rr * P, q0_dve, k % 2 == 1)
                        )
                        k += 1
                for y_a, row_a, qd_a, f8_a in pend:
                    zt_a = pass_b1(y_a)
                    pass_b2(zt_a, row_a, qd_a, f8_a)

            if repeat > 1:
                # Hardware loop: program size stays constant so large
                # repeat counts (for slope timing) compile fast.
                with tc.For_i(0, repeat):
                    full_pass()
            else:
                full_pass()

    nc.compile()
    return nc


def _get_nc(repeat: int = 1):
    if repeat not in _NC:
        _NC[repeat] = _build_nc(repeat)
    return _NC[repeat]


def prepare_in_maps(x, angles):
    WA, WB = _build_weights(angles)
    xdt = (
        ml_dtypes.float8_e3m4
        if _XDT in ("f8", "f8sb")
        else ml_dtypes.bfloat16
    )
    xb = x.astype(xdt)
    # [core, b, i, h, r] = x[core*4096 + b*256 + r, h*128 + i]: each
    # (b) is one contiguous load block on the device.
    tiled = np.ascontiguousarray(
        xb.reshape(N_CORES, N_BLK, R_BLK, N_CHUNKS, P).transpose(
            0, 1, 4, 3, 2
        )
    )
    return [
        {"x": tiled[i], "wa": WA, "wb": WB}
        for i in range(N_CORES)
    ]


def unpermute(out_dev: np.ndarray) -> np.ndarray:
    """Device col (g, hp, j16) -> true col (hp, g, j16)."""
    b = out_dev.shape[0]
    return np.ascontiguousarray(
        out_dev.reshape(b, 8, 8, 16).transpose(0, 2, 1, 3).reshape(b, DIM)
    )


def host_ref(x, angles):
    B = _stage_product(angles, range(STAGES))
    return x.astype(np.float64) @ B.T


def kernel(x: np.ndarray, angles: np.ndarray) -> np.ndarray:
    x = np.ascontiguousarray(np.asarray(x, dtype=np.float32))
    angles = np.asarray(angles, dtype=np.float32)
    assert x.shape == (BATCH, DIM), x.shape

    in_maps = prepare_in_maps(x, angles)

    nc = _get_nc()
    res = run_bass_kernel_spmd(nc, in_maps, list(range(N_CORES)))

    def core_out(i):
        r = res.results[i]
        if _ODT == "rsplit":
            dev = np.empty((ROWS_PER_CORE, DIM), dtype=np.float32)
            d4 = dev.reshape(ROWS_PER_CORE // P // 2, 2, P, DIM)
            d4[:, 0] = r["out"].astype(np.float32).reshape(-1, P, DIM)
            d4[:, 1] = r["out8"].astype(np.float32).reshape(-1, P, DIM)
        elif _ODT == "split":
            dev = np.empty((ROWS_PER_CORE, DIM), dtype=np.float32)
            dev[:, : DIM // 2] = r["out"].astype(np.float32)
            dev[:, DIM // 2 :] = r["out8"].astype(np.float32)
        else:
            dev = r["out"].astype(np.float32)
        return unpermute(dev)

    return np.concatenate([core_out(i) for i in range(N_CORES)], axis=0)



# revision 72
# speedup vs baseline: 34.1175x; 1.6488x over previous
"""Structured butterfly kernel, bf16 datapath (fp32 PSUM accumulation).

x is cast to bf16 AND pre-transposed on host, so the device does only
contiguous DMA loads.  Stages 0-6 (128x128 block-diag) run as
data-stationary bf16 matmuls into one 2-bank PSUM tile per subtile, so
the y drain is a single merged scatter copy; stages 7-9 as a PE
transpose pass + bf16 matmuls against 16x block-diag(8x8) weights.
Drain assignment (HW-measured best): y and o.q0 alternate ACT/DVE by
subtile parity, the z drain is pinned to DVE (bf16 runs 2x there, 1x
on ACT's InstActivation), and o.q1 is pinned to ACT as payback.  Loads
ride the gpsimd SWDGE queue, stores the SP ring.  The output leaves
the device in (g, hp, j16) column order; the host un-permutes with a
cheap reshape/transpose and upcasts bf16 -> f32.
"""

import os

import numpy as np
import ml_dtypes

import concourse.bacc as bacc
import concourse.mybir as mybir
import concourse.tile as tile
from concourse.bass_utils import run_bass_kernel_spmd
from concourse.masks import make_identity

N_CORES = 8
BATCH = 32768
DIM = 1024
STAGES = 10
P = 128
ROWS_PER_CORE = BATCH // N_CORES          # 4096
R_SUPER = 1024                            # rows per load DMA
N_SUPER = ROWS_PER_CORE // R_SUPER        # 4
N_TILES = ROWS_PER_CORE // P              # 32
N_CHUNKS = DIM // P                       # 8
R_BLK = int(__import__("os").environ.get("K_RBLK", "1024"))
N_BLK = ROWS_PER_CORE // R_BLK            # load blocks per core
F32 = mybir.dt.float32
BF16 = mybir.dt.bfloat16

_NC = {}

# Drain routing (subtile-index-mod-8 sets), overridable for A/B benching.
_Y_DVE = tuple(
    int(c) for c in os.environ.get("K_Y_DVE", "0246")
)
_Q0_DVE = tuple(
    int(c) for c in os.environ.get("K_Q0_DVE", "135")
)
_LOADSPLIT = os.environ.get("K_LOADSPLIT", "1") == "1"
# Timing-only probes: drop one component to locate the HW critical path.
# full | nostore | noload | nodrain | nope
_PROBE = os.environ.get("K_PROBE", "full")
# Load DMA ring: gp = gpsimd SWDGE, act = qActDynamicHW, sp = qSPDynamicHW
# (must be gp when _XDT casts: only SWDGE DMAs can convert dtypes)
_LOADQ = os.environ.get("K_LOADQ", "gp")
# DRAM dtype of x: f8 = float8_e3m4 cast to bf16 in the load DMA;
# f8sb = float8_e3m4 end-to-end (fp8 in SBUF, MMA reads fp8 moving
# operand directly — halves both DRAM AND SBUF-write traffic);
# bf16 = as-is.  f8 variants add ~1.3% input quantization error.
_XDT = os.environ.get("K_XDT", "f8sb")
# Output DRAM dtype: rsplit = even subtiles bf16 + odd subtiles e3m4
# (row split: full-width DMAs, total err ~1.7%); split = column split
# (slow: halves store descriptor sizes); bf16 = all bf16; f8cast
# (probe only: all e3m4, exceeds the error budget)
# NOTE: e3m4 engine-writes are broken on TRN2 HW (DVE wedges the core,
# ACT is ~4x slow) — only bf16 output is viable.
_ODT = os.environ.get("K_ODT", "bf16")
# Store batching: subtiles per store DMA (1 or 2)
_SBATCH = int(os.environ.get("K_SBATCH", "1"))


def _stage_product(angles: np.ndarray, stages) -> np.ndarray:
    B = np.eye(DIM, dtype=np.float64)
    k = np.arange(DIM)
    for s in stages:
        stride = 1 << s
        b = k // (2 * stride)
        j = k % stride
        h = (k >> s) & 1
        th = angles[s].astype(np.float64)[b * stride + j]
        C = np.cos(th)
        S = np.where(h == 0, -np.sin(th), np.sin(th))
        B = C[:, None] * B + S[:, None] * B[k ^ stride]
    return B


def _build_weights(angles: np.ndarray):
    """Returns (WA [128, 1024], WB [128, 1024]) bf16.

    WA: per-chunk transposed stage-0..6 product (y keeps natural
    column order h*128 + c).
    WB: for zt partition order (h, j16) of group g (c = g*16 + j16),
    moving columns in (hp, j16) order.
    """
    B_lo = _stage_product(angles, range(7))
    B_hi = _stage_product(angles, range(7, 10))

    WA = np.zeros((P, N_CHUNKS * P), dtype=np.float64)
    for h in range(N_CHUNKS):
        blk = B_lo[h * P:(h + 1) * P, h * P:(h + 1) * P]
        WA[:, h * P:(h + 1) * P] = blk.T

    jj = np.arange(P)
    H = np.zeros((P, 8, 8), dtype=np.float64)
    for hp in range(8):
        for h in range(8):
            H[:, hp, h] = B_hi[hp * P + jj, h * P + jj]

    WB = np.zeros((P, N_CHUNKS * P), dtype=np.float64)
    for g in range(8):
        blk = np.zeros((P, P), dtype=np.float64)
        for j16 in range(16):
            j = 16 * g + j16
            for h in range(8):
                for hp in range(8):
                    blk[h * 16 + j16, hp * 16 + j16] = H[j, hp, h]
        WB[:, g * P:(g + 1) * P] = blk
    return (WA.astype(ml_dtypes.bfloat16), WB.astype(ml_dtypes.bfloat16))


def _build_nc(repeat: int = 1):
    nc = bacc.Bacc(
        "TRN2", target_bir_lowering=False, debug=False, num_devices=N_CORES
    )
    # x arrives pre-transposed AND load-tiled on host:
    # x_in[b, i, h, r] = x[b*R_BLK + r, h*128 + i], so each load DMA
    # reads one fully contiguous 512KB DRAM block (4KB runs per
    # partition, ascending addresses — DRAM-sequential).
    F8 = mybir.dt.float8e3
    x_in = nc.dram_tensor(
        "x",
        [N_BLK, P, N_CHUNKS, R_BLK],
        F8 if _XDT in ("f8", "f8sb") else BF16,
        kind="ExternalInput",
    ).ap()
    XT_SB = F8 if _XDT == "f8sb" else BF16
    wa_in = nc.dram_tensor("wa", [P, DIM], BF16, kind="ExternalInput").ap()
    wb_in = nc.dram_tensor("wb", [P, DIM], BF16, kind="ExternalInput").ap()
    if _ODT == "rsplit":
        out = nc.dram_tensor(
            "out", [ROWS_PER_CORE // 2, DIM], BF16, kind="ExternalOutput"
        ).ap()
        out8 = nc.dram_tensor(
            "out8", [ROWS_PER_CORE // 2, DIM], F8, kind="ExternalOutput"
        ).ap()
    elif _ODT == "split":
        out = nc.dram_tensor(
            "out", [ROWS_PER_CORE, DIM // 2], BF16, kind="ExternalOutput"
        ).ap()
        out8 = nc.dram_tensor(
            "out8", [ROWS_PER_CORE, DIM // 2], F8, kind="ExternalOutput"
        ).ap()
    else:
        out = nc.dram_tensor(
            "out",
            [ROWS_PER_CORE, DIM],
            F8 if _ODT == "f8cast" else BF16,
            kind="ExternalOutput",
        ).ap()
        out8 = None



    with tile.TileContext(nc) as tc:
        from contextlib import ExitStack

        with ExitStack() as ctx:
            const = ctx.enter_context(tc.tile_pool(name="const", bufs=1))
            ident = const.tile([P, P], BF16)
            make_identity(nc, ident)

            # 4+2+2 = 8 banks: mma tiles span 2 banks each so all 8
            # MM-A matmuls share one tile and y drains in a single op.
            mma = ctx.enter_context(
                tc.tile_pool(name="mma", bufs=2, space="PSUM")
            )
            tp2 = ctx.enter_context(
                tc.tile_pool(name="tp2", bufs=2, space="PSUM")
            )
            mmb = ctx.enter_context(
                tc.tile_pool(name="mmb", bufs=2, space="PSUM")
            )

            # Consume identity on PE early (single-wait discipline) and
            # trigger the ACT function-table load during startup.
            warm = tp2.tile([P, P], BF16, tag="pt2")
            nc.tensor.transpose(warm[:], ident[:], ident[:])
            warm_act = const.tile([P, 1], BF16)
            nc.scalar.copy(out=warm_act[:], in_=ident[:, 0:1])

            wa_sb = const.tile([P, DIM], BF16)
            nc.sync.dma_start(wa_sb[:], wa_in[:])
            wb_sb = const.tile([P, DIM], BF16)
            nc.sync.dma_start(wb_sb[:], wb_in[:])

            xt_pool = ctx.enter_context(tc.tile_pool(name="xt", bufs=3))
            y_pool = ctx.enter_context(tc.tile_pool(name="y", bufs=4))
            zt_pool = ctx.enter_context(tc.tile_pool(name="zt", bufs=4))
            o_pool = ctx.enter_context(tc.tile_pool(name="o", bufs=4))
            o8_pool = ctx.enter_context(tc.tile_pool(name="o8", bufs=4))

            H4 = P * 4
            o_hold = [None]  # paired store tile (see _SBATCH)

            def cp(use_dve, out_ap, in_ap):
                if _PROBE == "nodrain":
                    # keep the tile allocated with a 1-col copy
                    nc.gpsimd.memset(out_ap[:, 0:1], 0.0)
                    return
                if use_dve:
                    nc.vector.tensor_copy(out=out_ap, in_=in_ap)
                else:
                    nc.scalar.copy(out=out_ap, in_=in_ap)

            def pass_a(xt, rbase, dve):
                """dve: route the y drain to DVE (else ACT)."""
                """MM-A for one subtile -> (g, h, j16)-ordered bf16 y.

                The h-regrouping scatter rides the y drain (matmul
                stationary APs must be 2D, so it cannot fold into T2).
                """
                y_t = y_pool.tile([P, DIM], BF16, tag="y")
                # Out dims iterated (g, h, j): h+j merge into 128-elem
                # contiguous write runs (vs 16-elem in (h, g, j) order).
                y_scatter = y_t[:].rearrange("p (g h j) -> p g h j", g=8, h=8)
                # 2-bank mma tile: all 8 matmuls land in one tile so the
                # y drain is a single merged op.
                bank_a = mma.tile([P, DIM], F32, tag="pa")
                nh = 1 if _PROBE == "nope" else 8
                for h in range(nh):
                    nc.tensor.matmul(
                        bank_a[:, h * P : (h + 1) * P],
                        xt[:, h, rbase : rbase + P],
                        wa_sb[:, h * P : (h + 1) * P],
                        start=True,
                        stop=True,
                    )
                cp(
                    dve,
                    y_scatter[:],
                    bank_a[:].rearrange("p (h g j) -> p g h j", h=8, g=8),
                )
                return y_t

            def pass_b1(y_t):
                """T2 (gather-AP folds the (h,j16) regroup) + z drain."""
                bank_t2 = tp2.tile([P, DIM], BF16, tag="pt2")
                zt_q = zt_pool.tile([P, DIM], BF16, tag="zt")
                ng = 1 if _PROBE == "nope" else 8
                for g in range(ng):
                    nc.tensor.transpose(
                        bank_t2[:, g * P : (g + 1) * P],
                        y_t[:, g * P : (g + 1) * P],
                        ident[:],
                    )
                # z always drains on DVE: bf16->bf16 runs at 2x there,
                # but at 1x on ACT's InstActivation.
                cp(True, zt_q[:], bank_t2[:])
                return zt_q

            def pass_b2(zt_q, row0, q0_dve, f8_row=False, k4=0):
                """MM-B; drains contiguous; stores permuted output.
                Under rsplit, f8_row subtiles drain+store as e3m4."""
                split = _ODT == "split"
                rsplit = _ODT == "rsplit"
                pair = _SBATCH == 2 and _ODT == "bf16"
                if rsplit and f8_row:
                    o_t = None
                    o_8 = o8_pool.tile([P, DIM], F8, tag="o8")
                elif split:
                    o_t = o_pool.tile([P, H4], BF16, tag="o")
                    o_8 = o8_pool.tile([P, H4], F8, tag="o8")
                elif pair:
                    # one [P, 2*DIM] tile holds 2 subtiles; stored in a
                    # single DMA after the odd subtile fills cols DIM:.
                    even = (row0 // P) % 2 == 0
                    if even:
                        o_hold[0] = o_pool.tile(
                            [P, 2 * DIM], BF16, tag="o", name="o_pair"
                        )
                    o_t = o_hold[0]
                    o_8 = None
                else:
                    o_t = o_pool.tile([P, DIM], BF16, tag="o")
                    o_8 = None
                for q in range(2):
                    bank_b = mmb.tile([P, H4], F32, tag="pb")
                    ngg = 1 if _PROBE == "nope" else 4
                    for gg in range(ngg):
                        g = 4 * q + gg
                        nc.tensor.matmul(
                            bank_b[:, gg * P : (gg + 1) * P],
                            zt_q[:, g * P : (g + 1) * P],
                            wb_sb[:, g * P : (g + 1) * P],
                            start=True,
                            stop=True,
                        )
                    # q0 occasionally rides DVE (LP-balanced share);
                    # q1 is pinned to ACT.  f8 rows: engine set by env
                    # (probing which engine converts to e3m4 faster).
                    if rsplit and f8_row:
                        o_dve = os.environ.get("K_F8ENG", "dve") == "dve"
                    else:
                        o_dve = q0_dve if q == 0 else False
                    if split:
                        dst = o_t[:] if q == 0 else o_8[:]
                    elif rsplit and f8_row:
                        dst = o_8[:, q * H4 : (q + 1) * H4]
                    elif pair:
                        ocol = 0 if even else DIM
                        dst = o_t[:, ocol + q * H4 : ocol + (q + 1) * H4]
                    else:
                        dst = o_t[:, q * H4 : (q + 1) * H4]
                    cp(o_dve, dst, bank_b[:])
                if _PROBE != "nostore":
                    if pair:
                        if not even:
                            row_e = row0 - P
                            dst2 = out[row_e : row_e + 2 * P, :].rearrange(
                                "(b p) c -> p b c", b=2
                            )
                            nc.sync.dma_start(dst2, o_t[:])
                    elif rsplit:
                        half_row = (row0 // (2 * P)) * P
                        if f8_row:
                            nc.sync.dma_start(
                                out8[half_row : half_row + P, :], o_8[:]
                            )
                        else:
                            nc.sync.dma_start(
                                out[half_row : half_row + P, :], o_t[:]
                            )
                    elif split:
                        nc.sync.dma_start(
                            out[row0 : row0 + P, :], o_t[:]
                        )
                        nc.sync.dma_start(
                            out8[row0 : row0 + P, :], o_8[:]
                        )
                    else:
                        store_eng = (
                            nc.gpsimd if _ODT == "f8cast" else nc.sync
                        )
                        store_eng.dma_start(
                            out[row0 : row0 + P, :], o_t[:]
                        )

            def load_engine(blk):
                if _LOADQ == "mix":
                    # alternate SWDGE / ACT-HWDGE rings per block
                    return nc.gpsimd if blk % 2 == 0 else nc.scalar
                return {
                    "gp": nc.gpsimd,
                    "act": nc.scalar,
                    "sp": nc.sync,
                }[_LOADQ]

            def load_supertile(st):
                """Load one supertile's xt via contiguous DRAM blocks."""
                xt = xt_pool.tile([P, N_CHUNKS, R_SUPER], XT_SB, tag="xt")
                b0 = st * (R_SUPER // R_BLK)
                nb = R_SUPER // R_BLK
                for j in range(nb):
                    load_engine(b0 + j).dma_start(
                        xt[:, :, j * R_BLK : (j + 1) * R_BLK],
                        x_in[b0 + j],
                    )
                return xt

            def dma_probe_pass():
                # loads and/or stores only — measures raw DMA rates.
                for st in range(N_SUPER):
                    r0 = st * R_SUPER
                    if _PROBE in ("dmaonly", "loadonly"):
                        load_supertile(st)
                    if _PROBE in ("dmaonly", "storeonly"):
                        for rr in range(R_SUPER // P):
                            row0 = r0 + rr * P
                            if _ODT == "rsplit":
                                half_row = (row0 // (2 * P)) * P
                                if (row0 // P) % 2 == 1:
                                    o_8 = o8_pool.tile(
                                        [P, DIM], F8, tag="o8"
                                    )
                                    nc.gpsimd.memset(o_8[:, 0:1], 0.0)
                                    nc.sync.dma_start(
                                        out8[half_row : half_row + P, :],
                                        o_8[:],
                                    )
                                else:
                                    o_t = o_pool.tile(
                                        [P, DIM], BF16, tag="o"
                                    )
                                    nc.gpsimd.memset(o_t[:, 0:1], 0.0)
                                    nc.sync.dma_start(
                                        out[half_row : half_row + P, :],
                                        o_t[:],
                                    )
                            elif _ODT == "split":
                                o_t = o_pool.tile([P, H4], BF16, tag="o")
                                o_8 = o8_pool.tile([P, H4], F8, tag="o8")
                                nc.gpsimd.memset(o_t[:, 0:1], 0.0)
                                nc.gpsimd.memset(o_8[:, 0:1], 0.0)
                                nc.sync.dma_start(
                                    out[row0 : row0 + P, :], o_t[:]
                                )
                                nc.sync.dma_start(
                                    out8[row0 : row0 + P, :], o_8[:]
                                )
                            else:
                                o_t = o_pool.tile([P, DIM], BF16, tag="o")
                                nc.gpsimd.memset(o_t[:, 0:1], 0.0)
                                store_eng = (
                                    nc.gpsimd
                                    if _ODT == "f8cast"
                                    else nc.sync
                                )
                                store_eng.dma_start(
                                    out[row0 : row0 + P, :], o_t[:]
                                )

            def full_pass():
                if _PROBE in ("dmaonly", "loadonly", "storeonly"):
                    return dma_probe_pass()
                # 1-deep software pipeline, pass_b(k-1) issued BEFORE
                # pass_a(k): engines are in-order, so the ready work
                # (T2/MM-B drains of subtile k-1) must enter each engine
                # stream ahead of ops gated on MM-A(k).  Drain routing
                # is the LP-balanced split: z always DVE (2x bf16), y on
                # DVE for even subtiles, q0 on DVE 3-of-8, q1 always ACT.
                k = 0
                pend = []
                for st in range(N_SUPER):
                    r0 = st * R_SUPER
                    if _PROBE == "noload":
                        xt = xt_pool.tile(
                            [P, N_CHUNKS, R_SUPER], XT_SB, tag="xt"
                        )
                        nc.gpsimd.memset(xt[:, 0, 0:1], 0.0)
                    else:
                        xt = load_supertile(st)
                    for rr in range(R_SUPER // P):
                        i8 = k % 8
                        y_dve = i8 in _Y_DVE
                        q0_dve = i8 in _Q0_DVE
                        if pend:
                            y_a, row_a, qd_a, f8_a = pend.pop(0)
                            zt_a = pass_b1(y_a)
                            y_t = pass_a(xt, rr * P, y_dve)
                            pass_b2(zt_a, row_a, qd_a, f8_a)
                        else:
                            y_t = pass_a(xt, rr * P, y_dve)
                        pend.append(
                            (y_t, r0 + rr * P, q0_dve, k % 2 == 1)
                        )
                        k += 1
                for y_a, row_a, qd_a, f8_a in pend:
                    zt_a = pass_b1(y_a)
                    pass_b2(zt_a, row_a, qd_a, f8_a)

            if repeat > 1:
                # Hardware loop: program size stays constant so large
                # repeat counts (for slope timing) compile fast.
                with tc.For_i(0, repeat):
                    full_pass()
            else:
                full_pass()

    nc.compile()
    return nc


def _get_nc(repeat: int = 1):
    if repeat not in _NC:
        _NC[repeat] = _build_nc(repeat)
    return _NC[repeat]


def prepare_in_maps(x, angles):
    WA, WB = _build_weights(angles)
    xdt = (
        ml_dtypes.float8_e3m4
        if _XDT in ("f8", "f8sb")
        else ml_dtypes.bfloat16
    )
    xb = x.astype(xdt)
    # [core, b, i, h, r] = x[core*4096 + b*256 + r, h*128 + i]: each
    # (b) is one contiguous load block on the device.
    tiled = np.ascontiguousarray(
        xb.reshape(N_CORES, N_BLK, R_BLK, N_CHUNKS, P).transpose(
            0, 1, 4, 3, 2
        )
    )
    return [
        {"x": tiled[i], "wa": WA, "wb": WB}
        for i in range(N_CORES)
    ]


def unpermute(out_dev: np.ndarray) -> np.ndarray:
    """Device col (g, hp, j16) -> true col (hp, g, j16)."""
    b = out_dev.shape[0]
    return np.ascontiguousarray(
        out_dev.reshape(b, 8, 8, 16).transpose(0, 2, 1, 3).reshape(b, DIM)
    )


def host_ref(x, angles):
    B = _stage_product(angles, range(STAGES))
    return x.astype(np.float64) @ B.T


def kernel(x: np.ndarray, angles: np.ndarray) -> np.ndarray:
    x = np.ascontiguousarray(np.asarray(x, dtype=np.float32))
    angles = np.asarray(angles, dtype=np.float32)
    assert x.shape == (BATCH, DIM), x.shape

    in_maps = prepare_in_maps(x, angles)

    nc = _get_nc()
    res = run_bass_kernel_spmd(nc, in_maps, list(range(N_CORES)))

    def core_out(i):
        r = res.results[i]
        if _ODT == "rsplit":
            dev = np.empty((ROWS_PER_CORE, DIM), dtype=np.float32)
            d4 = dev.reshape(ROWS_PER_CORE // P // 2, 2, P, DIM)
            d4[:, 0] = r["out"].astype(np.float32).reshape(-1, P, DIM)
            d4[:, 1] = r["out8"].astype(np.float32).reshape(-1, P, DIM)
        elif _ODT == "split":
            dev = np.empty((ROWS_PER_CORE, DIM), dtype=np.float32)
            dev[:, : DIM // 2] = r["out"].astype(np.float32)
            dev[:, DIM // 2 :] = r["out8"].astype(np.float32)
        else:
            dev = r["out"].astype(np.float32)
        return unpermute(dev)

    return np.concatenate([core_out(i) for i in range(N_CORES)], axis=0)

